# revision 12
# baseline (speedup 1.0000x reference)
"""Trainium2 Bass kernel for nn_Attention_27797028340174.

Multi-head attention, B=4, S=2048, H=16 heads, D=64 (HID=1024):
    x = query.reshape(B*S, HID)                     (the `key` input is
    q,k,v = x@Wq+bq, x@Wk+bk, x@Wv+bv                ignored: source bug
    per (b,h): softmax(q k^T / 8) @ v                makes k,v from query)

Sharding: tensor-parallel over the 16 heads -> 2 heads per NeuronCore,
zero collectives. Each core receives the full transposed activations
xT = x.T (bf16) plus its 128-column slice of Wq/Wk/Wv, and produces its
[8192, 128] slice of the output; the host concatenates slices.

v3 design notes. The steady state is paced by the Scalar engine
(exp over 4M scores per (b,h) pair); everything else hides under it:
  - scores PSUM: one [128,2048] (4 banks) + one [128,1024] (2 banks)
    tile. Even j-tiles get a single 2048-wide EXP activation, odd
    j-tiles two 1024-wide ones -> 24 ACTs/pair instead of 32.
  - one shared [128,512] aux PSUM pool (2 banks, double buffered) for
    projection chains, v chains and ctx accumulation chains; the
    rotation guarantees back-to-back chains never wait on the DVE
    drain of the previous chain.
  - PE work is emitted in small units (ctx quartets, proj halves)
    popped between scores matmuls; PE idle slices stay well under 1us
    so the PE clock never de-ramps (all DMAs ride the Sync HW queue --
    the GpSimd software queue is ~10x slower, measured).
  - epilogue: the last pair's ctx is staged (j-tiles 0-7, then 8-11,
    then 12-15 after the final exp) so the tail after the last ACT is
    ~3us instead of ~18us.

Device algorithm per core (all matmuls bf16, fp32 PSUM):
  qT,kT = W.T @ xT        [64, 4096] per-head column blocks
  v     = xT.T @ Wv       [seq, 128] natural layout (+ ones column)
  per (b,h), per jt:  scoresT[j,i] = kT_tile.T @ qT  (K=64)
      expT = exp(scoresT/8)  (ScalarE, bf16 out)
  per i-group of 4 tiles: ctx, Z = expT.T @ [v | 1]  (K=128 chains)
      out = ctx * reciprocal(Z)    (VectorE)

Assumptions hard-verified on host: attention_mask all ones (mask term
== 0), zero biases. These hold for the problem's setup_inputs().
"""

from contextlib import ExitStack

import numpy as np
import ml_dtypes

import concourse.bass as bass
import concourse.tile as tile
from concourse import bacc, mybir
from concourse.bass_utils import run_bass_kernel_spmd

BF16 = mybir.dt.bfloat16
F32 = mybir.dt.float32

B = 4  # batches
S = 2048  # seq per batch
HID = 1024
NCORES = 8
NH = 2  # heads per core
D = 64
KT = 8  # hid tiles of 128
JT = 16  # key tiles of 128 per batch
IT = 16  # query tiles of 128 per batch
CH = 4  # seq chunks of 512 per batch
CW = 512  # chunk width

EXP_BUFS = 30
XT_BUFS = 3

_CACHE = {}


def _build():
    nc = bacc.Bacc(
        "TRN2", target_bir_lowering=False, debug=False, num_devices=NCORES
    )
    xt = nc.dram_tensor("xt", [HID, B * S], BF16, kind="ExternalInput")
    wq = nc.dram_tensor("wq", [HID, 128], BF16, kind="ExternalInput")
    wk = nc.dram_tensor("wk", [HID, 128], BF16, kind="ExternalInput")
    wv = nc.dram_tensor("wv", [HID, 128], BF16, kind="ExternalInput")
    out = nc.dram_tensor("out", [B * S, 128], F32, kind="ExternalOutput")

    xt_v = xt.ap().rearrange("(kt p) n -> p kt n", p=128)  # [128, 8, 8192]
    out_v = out.ap().rearrange("(b it p) c -> p b it c", it=IT, p=128)

    with tile.TileContext(nc) as tc, ExitStack() as ctx:
        wp = ctx.enter_context(tc.tile_pool(name="w", bufs=1))
        xp = ctx.enter_context(tc.tile_pool(name="x", bufs=XT_BUFS))
        qkp = ctx.enter_context(tc.tile_pool(name="qk", bufs=2))
        ep = ctx.enter_context(tc.tile_pool(name="e", bufs=EXP_BUFS))
        op = ctx.enter_context(tc.tile_pool(name="o", bufs=2))
        zp = ctx.enter_context(tc.tile_pool(name="z", bufs=4))
        # 8 PSUM banks: scores wide [128,2048] (4) + narrow [128,1024] (2)
        # + proj/v chain bank (1) + ctx chain bank (1).  ctx chains stay
        # open across other units, and a chain start= clears its whole
        # 2KB bank -- so ctx must NEVER share a bank with proj/v chains.
        psW = ctx.enter_context(tc.tile_pool(name="psW", bufs=1, space="PSUM"))
        psN = ctx.enter_context(tc.tile_pool(name="psN", bufs=1, space="PSUM"))
        psq = ctx.enter_context(tc.tile_pool(name="psq", bufs=1, space="PSUM"))
        psc = ctx.enter_context(tc.tile_pool(name="psc", bufs=1, space="PSUM"))

        wq_sb = wp.tile([128, KT, 128], BF16)
        nc.sync.dma_start(wq_sb[:], wq.ap().rearrange("(kt p) m -> p kt m", p=128))
        wk_sb = wp.tile([128, KT, 128], BF16)
        wv_sb = wp.tile([128, KT, 128], BF16)
        w_loaded = {"k": False, "v": False}

        def load_w(which):
            if not w_loaded[which]:
                src = wk if which == "k" else wv
                dst = wk_sb if which == "k" else wv_sb
                nc.sync.dma_start(
                    dst[:], src.ap().rearrange("(kt p) m -> p kt m", p=128)
                )
                w_loaded[which] = True

        state = {}

        def alloc_batch(b):
            st = {}
            st["qT"] = qkp.tile([64, NH * S], BF16, tag="qT", name="qT")
            st["kT"] = qkp.tile([64, NH * S], BF16, tag="kT", name="kT")
            st["v"] = qkp.tile([128, JT, NH, D + 1], BF16, tag="v", name="v")
            nc.vector.memset(st["v"][:, :, :, D], 1.0)
            state[b] = st

        # ---- projection emitters ----
        def _proj_copyout(b, ch, ps, dst):
            nc.vector.tensor_copy(
                out=dst[:, ch * CW : (ch + 1) * CW], in_=ps[0:64, :]
            )
            stg = xp.tile([128, CW], BF16, tag="stg", name="stg")
            nc.vector.tensor_copy(out=stg[64:128, :], in_=ps[64:128, :])
            nc.sync.dma_start(
                dst[:, S + ch * CW : S + (ch + 1) * CW], stg[64:128, :]
            )

        def load_xt(b, ch):
            gc = b * CH + ch
            xt_t = xp.tile([128, KT, CW], BF16, tag="xt", name="xt_t")
            nc.sync.dma_start(xt_t[:], xt_v[:, :, gc * CW : (gc + 1) * CW])
            return xt_t

        def emit_proj(b, ch, which, xt_t):
            """Full q or k projection chunk (8 matmuls + copyout)."""
            if which == "k":
                load_w("k")
            w_sb = wq_sb if which == "q" else wk_sb
            ps = psq.tile([128, CW], F32, tag="aux", name="ps_qk")
            for kt in range(KT):
                nc.tensor.matmul(
                    ps[:],
                    lhsT=w_sb[:, kt],
                    rhs=xt_t[:, kt],
                    start=(kt == 0),
                    stop=(kt == KT - 1),
                )
            _proj_copyout(b, ch, ps, state[b]["qT" if which == "q" else "kT"])

        def emit_v_half(b, ch, half, xt_t):
            load_w("v")
            st = state[b]
            ps_full = psq.tile([128, CW], F32, tag="aux", name="ps_v")
            ps = ps_full[:, 0:256]
            for si, sub in enumerate((2 * half, 2 * half + 1)):
                for kt in range(KT):
                    nc.tensor.matmul(
                        ps[:, si * 128 : (si + 1) * 128],
                        lhsT=xt_t[:, kt, sub * 128 : (sub + 1) * 128],
                        rhs=wv_sb[:, kt],
                        start=(si == 0 and kt == 0),
                        stop=(si == 1 and kt == KT - 1),
                    )
            nc.vector.tensor_copy(
                out=st["v"][:, ch * 4 + 2 * half : ch * 4 + 2 * half + 2, :, 0:D],
                in_=ps[:].rearrange("p (s h d) -> p s h d", s=2, h=NH),
            )

        # ---- scores emitters ----
        def _smm(p, jt, ic, ps_dst):
            b, h = divmod(p, NH)
            st = state[b]
            nc.tensor.matmul(
                ps_dst,
                lhsT=st["kT"][:, h * S + jt * 128 : h * S + (jt + 1) * 128],
                rhs=st["qT"][:, h * S + ic * CW : h * S + (ic + 1) * CW],
                start=True,
                stop=True,
            )

        def _act(e_slice, ps_slice):
            nc.scalar.activation(
                e_slice, ps_slice, mybir.ActivationFunctionType.Exp, scale=0.125
            )

        # ---- ctx emitters ----
        def ctx_quartet(p, it0, exps, jts, cstate):
            """4 j-tiles x 4 q-slices of an accumulation chain."""
            b, h = divmod(p, NH)
            st = state[b]
            W = D + 1
            if cstate["ps"] is None:
                cstate["ps"] = psc.tile([128, 4 * W], F32, tag="ctx", name="ps_c")
            ps = cstate["ps"]
            for jt in jts:
                for q in range(4):
                    cstate["n"] += 1
                    nc.tensor.matmul(
                        ps[:, q * W : (q + 1) * W],
                        lhsT=exps[jt][:, (it0 + q) * 128 : (it0 + q + 1) * 128],
                        rhs=st["v"][:, jt, h],
                        start=(cstate["n"] == 1),
                        stop=(cstate["n"] == cstate["tot"]),
                    )
            return ps

        def ctx_norm(ps, it0, o_sb):
            W = D + 1
            rz = zp.tile([128, 4], F32, tag="rz", name="rz")
            z_view = ps[:].rearrange("p (q w) -> p q w", w=W)[:, :, D]
            nc.vector.reciprocal(rz[:], z_view)
            for q in range(4):
                nc.vector.tensor_scalar_mul(
                    o_sb[:, it0 + q], ps[:, q * W : q * W + D], rz[:, q : q + 1]
                )

        def emit_out_dma(p, o_sb, groups=(0, 1, 2, 3)):
            b, h = divmod(p, NH)
            for g in groups:
                nc.sync.dma_start(
                    out_v[:, b, 4 * g : 4 * g + 4, h * D : (h + 1) * D],
                    o_sb[:, 4 * g : 4 * g + 4],
                )

        # =================== prologue ===================
        warm_src = wp.tile([128, 1], F32, name="warm_src")
        warm_dst = wp.tile([128, 1], F32, name="warm_dst")
        nc.vector.memset(warm_src[:], 0.0)
        nc.scalar.activation(
            warm_dst[:], warm_src[:], mybir.ActivationFunctionType.Exp
        )
        # PE clock ramp warmup while the first DMAs fly
        warm_w = wp.tile([128, 64], BF16, name="warm_w")
        nc.vector.memset(warm_w[:], 0.0)
        warm_ps_full = psq.tile([128, CW], F32, tag="aux", name="warm_ps")
        warm_ps = warm_ps_full[:, 0:64]
        NWARM = 64
        for r in range(NWARM):
            nc.tensor.matmul(
                warm_ps[0:64, :],
                lhsT=warm_w[:],
                rhs=warm_w[:],
                start=(r == 0),
                stop=(r == NWARM - 1),
            )
        warm_junk = wp.tile([128, 64], F32, name="warm_junk")
        nc.vector.tensor_copy(out=warm_junk[0:64, :], in_=warm_ps[0:64, :])

        alloc_batch(0)
        exps0 = [None] * JT

        def _e0(jt):
            if exps0[jt] is None:
                exps0[jt] = ep.tile([128, S], BF16, tag="e", name="e0")
            return exps0[jt]

        pro_alt = {"i": 0}

        def s1024(jt, ihalf):
            """Prologue scores: one 1024-col half of a j-tile (2 mm + ACT)."""
            e = _e0(jt)
            if pro_alt["i"] % 2 == 0:
                ps = psW.tile([128, 2048], F32, tag="sW", name="ps_w")
            else:
                ps = psN.tile([128, 1024], F32, tag="sN", name="ps_n")
            pro_alt["i"] += 1
            for ic in range(2):
                _smm(0, jt, 2 * ihalf + ic, ps[:, ic * CW : (ic + 1) * CW])
            _act(e[:, ihalf * 1024 : (ihalf + 1) * 1024], ps[:, 0:1024])

        # batch-0 projections laddered with pair-0 scores
        xt_t = load_xt(0, 0)
        emit_proj(0, 0, "q", xt_t)
        emit_proj(0, 0, "k", xt_t)
        xt_t = load_xt(0, 1)
        emit_proj(0, 1, "q", xt_t)
        for jt in range(4):
            s1024(jt, 0)
        emit_proj(0, 1, "k", xt_t)
        for jt in range(4, 8):
            s1024(jt, 0)
        xt_t = load_xt(0, 2)
        emit_proj(0, 2, "q", xt_t)
        emit_proj(0, 2, "k", xt_t)
        for jt in range(8, 12):
            s1024(jt, 0)
        xt_t = load_xt(0, 3)
        emit_proj(0, 3, "q", xt_t)
        emit_proj(0, 3, "k", xt_t)
        load_w("v")
        xtv0 = load_xt(0, 0)
        for jt in range(12, JT):
            s1024(jt, 0)
        # second i-half + v(0) + qk(1) chunk0 interleaved
        xtv1 = load_xt(0, 1)
        for jt in range(0, 4):
            s1024(jt, 1)
        emit_v_half(0, 0, 0, xtv0)
        for jt in range(4, 6):
            s1024(jt, 1)
        emit_v_half(0, 0, 1, xtv0)
        xtv2 = load_xt(0, 2)
        for jt in range(6, 8):
            s1024(jt, 1)
        emit_v_half(0, 1, 0, xtv1)
        for jt in range(8, 10):
            s1024(jt, 1)
        emit_v_half(0, 1, 1, xtv1)
        xtv3 = load_xt(0, 3)
        for jt in range(10, 12):
            s1024(jt, 1)
        emit_v_half(0, 2, 0, xtv2)
        for jt in range(12, 14):
            s1024(jt, 1)
        emit_v_half(0, 2, 1, xtv2)
        xtq1 = load_xt(1, 0)
        for jt in range(14, JT):
            s1024(jt, 1)
        emit_v_half(0, 3, 0, xtv3)
        emit_v_half(0, 3, 1, xtv3)
        alloc_batch(1)
        emit_proj(1, 0, "q", xtq1)
        emit_proj(1, 0, "k", xtq1)
        prev = (0, exps0)

        # =================== steady pairs ===================
        NP = B * NH
        cst_tiles = []

        def build_units(p):
            """PE work units (est_ns, fn, min_jt) for pair p's slots.
            min_jt gates units whose dependencies are only emitted at a
            later scores slot (PE is in-order: popping too early would
            block the scores stream behind an unsatisfiable wait)."""
            b, h = divmod(p, NH)
            ctxu = []
            prju = []
            pp, pexps = prev
            o_prev = op.tile([128, IT, D], F32, tag="o", name="o_sb")

            # ctx chains for previous pair: 4 chains x 4 quartets
            for g in range(4):
                cstate = {"ps": None, "n": 0, "tot": 64}
                for jq in range(4):
                    def qfn(g=g, jq=jq, cstate=cstate):
                        ctx_quartet(
                            pp, 4 * g, pexps, range(4 * jq, 4 * jq + 4), cstate
                        )
                    ctxu.append((520, qfn, 0))

                def nfn(g=g, cstate=cstate, o_prev=o_prev):
                    ctx_norm(cstate["ps"], 4 * g, o_prev)
                ctxu.append((40, nfn, 0))

            def dfn(pp=pp, o_prev=o_prev):
                emit_out_dma(pp, o_prev)
            ctxu.append((20, dfn, 0))

            # projection duties (xt DMA prefetched ~2 units ahead)
            carry = {}

            def mk_ld(bb, chn, key):
                def ld(bb=bb, chn=chn, key=key):
                    carry[key] = load_xt(bb, chn)
                return ld

            if h == 0 and b >= 1:
                # v(b) all chunks + qk(b+1) chunk0
                prju.append((30, mk_ld(b, 0, "v0"), 0))
                prju.append((30, mk_ld(b, 1, "v1"), 0))
                plan = [(0, 0), (0, 1), "l2", (1, 0), (1, 1), "l3",
                        (2, 0), "lq", (2, 1), (3, 0), (3, 1)]
                for item in plan:
                    if item == "l2":
                        prju.append((30, mk_ld(b, 2, "v2"), 0))
                    elif item == "l3":
                        prju.append((30, mk_ld(b, 3, "v3"), 0))
                    elif item == "lq":
                        if b + 1 < B:
                            def ldq(b=b, carry=carry):
                                alloc_batch(b + 1)
                                carry["q0"] = load_xt(b + 1, 0)
                            prju.append((30, ldq, 0))
                    else:
                        chn, half = item
                        def vh(chn=chn, half=half, b=b, carry=carry):
                            emit_v_half(b, chn, half, carry[f"v{chn}"])
                        prju.append((890, vh, 0))
                if b + 1 < B:
                    def qk0(b=b, carry=carry):
                        emit_proj(b + 1, 0, "q", carry["q0"])

                    def qk0k(b=b, carry=carry):
                        emit_proj(b + 1, 0, "k", carry["q0"])
                    prju.append((1760, qk0, 0))
                    prju.append((1740, qk0k, 0))
            elif h == 1 and b + 1 < B:
                prju.append((30, mk_ld(b + 1, 1, "c1"), 0))
                for chn in range(1, CH):
                    def qc(b=b, chn=chn, carry=carry):
                        emit_proj(b + 1, chn, "q", carry[f"c{chn}"])
                    prju.append((1760, qc, 0))
                    if chn + 1 < CH:
                        prju.append((30, mk_ld(b + 1, chn + 1, f"c{chn + 1}"), 0))

                    def kc(b=b, chn=chn, carry=carry):
                        emit_proj(b + 1, chn, "k", carry[f"c{chn}"])
                    prju.append((1740, kc, 0))

            # weighted merge: spread proj chains through the ctx stream so
            # at least ~1 ctx quartet separates consecutive psq chains
            # (psq is single-banked: a chain start must wait the previous
            # chain's DVE copyout -- the interleaved ctx work covers it)
            units = []
            tc_, tp_ = sum(u[0] for u in ctxu), sum(u[0] for u in prju)
            ci = pi = 0
            sc = sp = 0
            while ci < len(ctxu) or pi < len(prju):
                take_proj = pi < len(prju) and (
                    ci >= len(ctxu)
                    or sp * (tc_ + 1) <= sc * (tp_ + 1)
                )
                if take_proj:
                    u = prju[pi]; pi += 1; sp += u[0]
                else:
                    u = ctxu[ci]; ci += 1; sc += u[0]
                units.append(u)
            return units, o_prev

        for p in range(1, NP):
            b, h = divmod(p, NH)
            exps = [
                ep.tile([128, S], BF16, tag="e", name="e") for _ in range(JT)
            ]
            units, o_prev = build_units(p)
            if p == NP - 1:
                # last pair: stage ctx for j-tiles 0..11 early; gated on
                # the slots that emit the needed exp ACTs
                for stage, jts, minj in ((0, range(0, 8), 8),
                                         (1, range(8, 12), 12)):
                    for g in range(4):
                        cstate = {"ps": None, "n": 0,
                                  "tot": 32 if stage == 0 else 16}

                        def sfn(g=g, stage=stage, jts=jts, cstate=cstate,
                                exps=exps, p=p):
                            ps = None
                            for jq in range(len(jts) // 4):
                                sub = list(jts)[4 * jq : 4 * jq + 4]
                                ps = ctx_quartet(p, 4 * g, exps, sub, cstate)
                            if stage == 0:
                                cst = op.tile(
                                    [128, 4 * (D + 1)], F32, tag="cst",
                                    bufs=4, name="cst",
                                )
                                nc.vector.tensor_copy(out=cst[:], in_=ps[:])
                                cst_tiles.append(cst)
                            else:
                                nc.vector.tensor_add(
                                    out=cst_tiles[g][:], in0=ps[:],
                                    in1=cst_tiles[g][:],
                                )
                        units.append(
                            (1120 if stage == 0 else 580, sfn, minj)
                        )

            uidx = {"i": 0}

            def pop_units(budget, cur_jt):
                spent = 0
                while uidx["i"] < len(units) and spent < budget:
                    est, fn, min_jt = units[uidx["i"]]
                    if min_jt > cur_jt:
                        break
                    fn()
                    spent += est
                    uidx["i"] += 1

            # budget the unit supply across the 24 pop points so the PE
            # never runs dry late in the pair (PE idle >~1us drops the
            # HAM fast-clock state); the odd-slot middle pop must cover
            # the preceding narrow ACT (~1.35us) or the second narrow
            # fill stalls the PE on the psN rotation.
            E = sum(u[0] for u in units)
            p1 = max(1400, int(E * 1.55 / 24))
            rem = max(0, E - 8 * p1)
            even_b = max(400, int(rem / 1.45 / 8))
            p2 = max(200, int(rem * 0.45 / 1.45 / 8))

            for jt in range(JT):
                e = exps[jt]
                if jt % 2 == 0:
                    psw = psW.tile([128, 2048], F32, tag="sW", name="ps_w")
                    for ic in range(4):
                        _smm(p, jt, ic, psw[:, ic * CW : (ic + 1) * CW])
                    _act(e[:, 0:2048], psw[:])
                    pop_units(even_b, jt)
                else:
                    psn = psN.tile([128, 1024], F32, tag="sN", name="ps_n")
                    for ic in range(2):
                        _smm(p, jt, ic, psn[:, ic * CW : (ic + 1) * CW])
                    _act(e[:, 0:1024], psn[:])
                    pop_units(p1, jt)
                    psn = psN.tile([128, 1024], F32, tag="sN", name="ps_n")
                    for ic in range(2, 4):
                        _smm(p, jt, ic, psn[:, (ic - 2) * CW : (ic - 1) * CW])
                    _act(e[:, 1024:2048], psn[:])
                    pop_units(p2, jt)
            # drain remaining units
            pop_units(1 << 30, JT)
            prev = (p, exps)

        # =================== epilogue ===================
        # last pair: j-tiles 12..15 + combine + normalize + out
        p, exps = prev
        o_last = op.tile([128, IT, D], F32, tag="o", name="o_last")
        for g in range(4):
            cstate = {"ps": None, "n": 0, "tot": 16}
            ps = ctx_quartet(p, 4 * g, exps, range(12, JT), cstate)
            nc.vector.tensor_add(
                out=cst_tiles[g][:], in0=ps[:], in1=cst_tiles[g][:]
            )
            ctx_norm(cst_tiles[g], 4 * g, o_last)
            emit_out_dma(p, o_last, groups=(g,))

    nc.compile()
    return nc


def _get_nc():
    if "nc" not in _CACHE:
        _CACHE["nc"] = _build()
    return _CACHE["nc"]


def kernel(
    query,
    key=None,
    attention_mask=None,
    Wq=None,
    bq=None,
    Wk=None,
    bk=None,
    Wv=None,
    bv=None,
    seq_length=2048,
    **_unused,
):
    query = np.asarray(query)
    Wq = np.asarray(Wq)
    Wk = np.asarray(Wk)
    Wv = np.asarray(Wv)
    if attention_mask is not None and not np.all(np.asarray(attention_mask) == 1):
        raise NotImplementedError("kernel assumes an all-ones attention mask")
    for bias in (bq, bk, bv):
        if bias is not None and np.any(np.asarray(bias)):
            raise NotImplementedError("kernel assumes zero biases")

    x = query.reshape(-1, HID)  # [8192, 1024]
    xt = np.ascontiguousarray(x.T).astype(ml_dtypes.bfloat16)  # [1024, 8192]

    in_maps = []
    for c in range(NCORES):
        cols = slice(c * 128, (c + 1) * 128)
        in_maps.append(
            {
                "xt": xt,
                "wq": np.ascontiguousarray(Wq[:, cols]).astype(ml_dtypes.bfloat16),
                "wk": np.ascontiguousarray(Wk[:, cols]).astype(ml_dtypes.bfloat16),
                "wv": np.ascontiguousarray(Wv[:, cols]).astype(ml_dtypes.bfloat16),
            }
        )

    nc = _get_nc()
    res = run_bass_kernel_spmd(
        nc,
        in_maps,
        core_ids=list(range(NCORES)),
        trace=bool(_CACHE.get("trace", False)),
    )
    _CACHE["last_result"] = res
    out = np.concatenate(
        [res.results[c]["out"] for c in range(NCORES)], axis=1
    ).astype(np.float32)
    return out
